# revision 11
# baseline (speedup 1.0000x reference)
"""Trainium2 Bass kernel for ConvexLORALinear: out = (input @ lora_A) @ lora_B.

Full shapes: input [8192, 4096] f32, lora_A [4096, 128] f32, lora_B [128, 4096] f32.
Sharding: data-parallel on the token dim — each of the 8 cores gets 1024 tokens,
lora_A / lora_B replicated. No collectives.

Per-core dataflow (all DMAs are natural/contiguous):
  1. input rows arrive as [128t, 4096k] tiles; the contraction dim (k) must sit on
     SBUF partitions for the PE, so each [128,128] block is transposed on the PE
     (transpose-mode matmul against an identity; exact data movement).
  2. mm1: C1T[r, t512] += A[kc].T @ inputT[kc, t512] accumulated over kc in PSUM,
     lhsT = A chunk (natural layout), rhs = transposed input, N=512.
  3. mm2: out[t128, n512] = C1T[:, t128].T @ B[:, n512] — both operands natural,
     single matmul per output tile (K = rank = 128), N=512.
Matmuls run as float32r (1 cycle/row at N>=512 vs 4 for plain float32).
"""

import os
import sys

import numpy as np

try:
    import concourse.bass as bass  # noqa: F401
except ImportError:  # concourse not on sys.path in this interpreter
    for _p in ("/opt/trn_rl_repo", os.path.expanduser("~/trn_rl_repo")):
        if os.path.isdir(_p) and _p not in sys.path:
            sys.path.insert(0, _p)
    import concourse.bass as bass

import concourse.mybir as mybir
from concourse.bass_utils import run_bass_kernel_spmd
from concourse.masks import make_identity
from concourse.tile import TileContext

P = 128
FREE = 512  # matmul moving-operand free dim (f32 PSUM bank = 512 floats)

N_CORES = 8
T_FULL = 8192
D_IN = 4096
RANK = 128
D_OUT = 4096

F32 = mybir.dt.float32


def _legalize_waits(nc: bass.Bass, cap: int = 1) -> None:
    """Split instructions carrying >cap semaphore waits.

    The walrus build in this environment rejects instructions with several
    sync-wait commands (seen on the TileContext tail drain: "Too many sync
    wait commands").  Hoist excess waits onto same-engine NOPs placed
    immediately before the instruction — the engine stream is serial, so
    waiting earlier on the same engine is equivalent.
    """
    n = 0
    for fn in nc.m.functions:
        for bb in fn.blocks:
            insts = bb.instructions
            new_list = []
            for inst in insts:
                si = inst.sync_info
                if si is not None and si.on_wait and len(si.on_wait) > cap:
                    waits = list(si.on_wait)
                    for w in waits[:-cap]:
                        nop = mybir.InstNoOp(
                            name=f"waitsplit-{inst.name}-{n}", ins=[], outs=[]
                        )
                        n += 1
                        nop.engine = inst.engine
                        nop.sync_info = mybir.SyncInfo(on_wait=[w], on_update=[])
                        new_list.append(nop)
                    inst.sync_info = mybir.SyncInfo(
                        on_wait=waits[-cap:], on_update=list(si.on_update or [])
                    )
                new_list.append(inst)
            insts[:] = new_list


def build_nc(
    t_core: int = T_FULL // N_CORES,
    d_in: int = D_IN,
    rank: int = RANK,
    d_out: int = D_OUT,
    mm_dt: mybir.dt = mybir.dt.float32r,
    legalize: bool = True,
) -> bass.Bass:
    assert t_core % FREE == 0 and d_in % P == 0 and d_out % FREE == 0
    assert rank == P, "kernel assumes rank == 128 (single contraction tile in mm2)"
    n_t_tiles = t_core // FREE  # 512-token slabs
    n_j = FREE // P  # 128-token blocks per slab
    n_kc = d_in // P  # contraction chunks for mm1
    n_nc = d_out // FREE  # output column chunks
    out_cols = min(d_out, 2048)  # SBUF output staging width per DMA
    n_halves = d_out // out_cols

    nc = bass.Bass()
    inp = nc.declare_dram_parameter("input", [t_core, d_in], F32, isOutput=False)
    a = nc.declare_dram_parameter("lora_A", [d_in, rank], F32, isOutput=False)
    b = nc.declare_dram_parameter("lora_B", [rank, d_out], F32, isOutput=False)
    outp = nc.declare_dram_parameter("output", [t_core, d_out], F32, isOutput=True)

    with TileContext(nc) as tc:
        with (
            tc.tile_pool(name="const", bufs=1) as const_pool,
            tc.tile_pool(name="a_sb", bufs=1) as a_pool,
            tc.tile_pool(name="b_sb", bufs=1) as b_pool,
            tc.tile_pool(name="nat", bufs=3) as nat_pool,
            tc.tile_pool(name="itp", bufs=n_kc + 2) as itp_pool,
            tc.tile_pool(name="c1t_sb", bufs=2) as c1t_pool,
            tc.tile_pool(name="out_sb", bufs=2) as out_pool,
            tc.tile_pool(name="tr_ps", bufs=4, space="PSUM") as tr_psum,
            tc.tile_pool(name="c1t_ps", bufs=2, space="PSUM") as c1t_psum,
            tc.tile_pool(name="out_ps", bufs=2, space="PSUM") as out_psum,
        ):
            identity = const_pool.tile([P, P], F32)
            make_identity(nc, identity)

            # A as [p, kc, r]: slice [:, kc, :] = A[kc*128:(kc+1)*128, :].
            # fp32r matmul operands must be produced pre-rounded to fp32r, so
            # DMA into an f32 staging tile and cast-copy into the fp32r tile.
            a_stage = a_pool.tile([P, n_kc, rank], F32, name="a_stage")
            nc.sync.dma_start(
                out=a_stage[:], in_=a.rearrange("(kc p) r -> p kc r", p=P)
            )
            a_sb = a_pool.tile([P, n_kc, rank], mm_dt, name="a_sb")
            nc.vector.tensor_copy(a_sb[:], a_stage[:])
            b_stage = b_pool.tile([P, d_out], F32, name="b_stage")
            nc.sync.dma_start(out=b_stage[:], in_=b[:, :])
            b_sb = b_pool.tile([P, d_out], mm_dt, name="b_sb")
            nc.scalar.copy(b_sb[:], b_stage[:])

            n_copy = 0  # alternation counter for DVE/ACT eviction balance

            def evict(dst, src):
                nonlocal n_copy
                if n_copy % 2 == 0:
                    nc.vector.tensor_copy(dst, src)
                else:
                    nc.scalar.copy(dst, src)
                n_copy += 1

            for tt in range(n_t_tiles):
                itps = [
                    itp_pool.tile([P, FREE], mm_dt, tag="itp", name=f"itp{tt}_{i}")
                    for i in range(n_kc)
                ]
                for j in range(n_j):
                    tb = tt * n_j + j
                    nat = nat_pool.tile([P, d_in], F32)
                    nc.sync.dma_start(out=nat[:], in_=inp[tb * P : (tb + 1) * P, :])
                    for kc in range(n_kc):
                        trp = tr_psum.tile([P, P], F32)
                        nc.tensor.matmul(
                            trp[:],
                            nat[:, kc * P : (kc + 1) * P],
                            identity[:],
                            is_transpose=True,
                            start=True,
                            stop=True,
                        )
                        evict(itps[kc][:, j * P : (j + 1) * P], trp[:])
                # mm1: C1T[r, t] accumulated over kc
                c1t_ps = c1t_psum.tile([P, FREE], F32)
                for kc in range(n_kc):
                    nc.tensor.matmul(
                        c1t_ps[:],
                        a_sb[:, kc, :],
                        itps[kc][:],
                        start=(kc == 0),
                        stop=(kc == n_kc - 1),
                    )
                c1t = c1t_pool.tile([P, FREE], mm_dt)
                nc.vector.tensor_copy(c1t[:, : FREE // 2], c1t_ps[:, : FREE // 2])
                nc.scalar.copy(c1t[:, FREE // 2 :], c1t_ps[:, FREE // 2 :])
                # mm2: out[t, n] = C1T[:, t].T @ B[:, n]
                for j in range(n_j):
                    tb = tt * n_j + j
                    for h in range(n_halves):
                        o_sb = out_pool.tile([P, out_cols], F32)
                        for q in range(n_nc // n_halves):
                            ncol = h * (n_nc // n_halves) + q
                            o_ps = out_psum.tile([P, FREE], F32)
                            nc.tensor.matmul(
                                o_ps[:],
                                c1t[:, j * P : (j + 1) * P],
                                b_sb[:, ncol * FREE : (ncol + 1) * FREE],
                                start=True,
                                stop=True,
                            )
                            evict(o_sb[:, q * FREE : (q + 1) * FREE], o_ps[:])
                        nc.sync.dma_start(
                            out=outp[
                                tb * P : (tb + 1) * P,
                                h * out_cols : (h + 1) * out_cols,
                            ],
                            in_=o_sb[:],
                        )
    if legalize:
        _legalize_waits(nc)
    return nc


_NC_CACHE: dict[tuple, bass.Bass] = {}


def _get_nc(**kw) -> bass.Bass:
    key = tuple(sorted(kw.items()))
    if key not in _NC_CACHE:
        _NC_CACHE[key] = build_nc(**kw)
    return _NC_CACHE[key]


def kernel(input: np.ndarray, lora_A: np.ndarray, lora_B: np.ndarray) -> np.ndarray:
    input = np.ascontiguousarray(np.asarray(input, dtype=np.float32))
    lora_A = np.ascontiguousarray(np.asarray(lora_A, dtype=np.float32))
    lora_B = np.ascontiguousarray(np.asarray(lora_B, dtype=np.float32))
    assert input.shape == (T_FULL, D_IN), input.shape
    assert lora_A.shape == (D_IN, RANK), lora_A.shape
    assert lora_B.shape == (RANK, D_OUT), lora_B.shape

    t_core = T_FULL // N_CORES
    shards = input.reshape(N_CORES, t_core, D_IN)
    nc = _get_nc()
    in_maps = [
        {"input": shards[i], "lora_A": lora_A, "lora_B": lora_B}
        for i in range(N_CORES)
    ]
    res = run_bass_kernel_spmd(nc, in_maps, list(range(N_CORES)))
    return np.concatenate(
        [res.results[i]["output"] for i in range(N_CORES)], axis=0
    )
